# revision 12
# baseline (speedup 1.0000x reference)
"""NoisyTopKGate Trainium2 kernel (8-core SPMD, data-parallel over tokens).

Computes, for x:[65536,1024] f32:
  logits      = x @ Wg_w.T + Wg_b          [T, 64]
  noise_scale = softplus(x @ Wn_w.T + Wn_b)
  H           = logits + noise * noise_scale
  top2 vals/idx over experts, softmax over top2, scatter into gates [T, 64]
Returns (gates, H, topk_idx(int32), noise_scale, logits) — full shapes.

Per-core plan (8192 tokens each):
  - supertile = 512 tokens; x tile [128p, 4, 1024] (token = st*512 + s*128 + p)
  - PE transposes x -> xT chunks [128 m, 512 t]; f32 matmul vs host-prepacked
    Wcat [1024, 128] (= [Wg_w.T | Wn_w.T]) accumulating into PSUM [128 ec, 512 t]
  - ACT adds bias during PSUM->SBUF copy; PE transposes back to [t, ec] layout
  - softplus = Ln(Exp(z)+1) on ACT; top-2 via DVE InstMax/InstMaxIndex
  - gates = Exp(H - max1 - ln(1+exp(max2-max1))) * (H >= max2)
"""

import sys

if "/opt/trn_rl_repo" not in sys.path:
    sys.path.insert(0, "/opt/trn_rl_repo")

import numpy as np

import concourse.bacc as bacc
import concourse.tile as tile
import concourse.mybir as mybir
from concourse import masks
from concourse.bass_utils import run_bass_kernel_spmd

F32 = mybir.dt.float32
U32 = mybir.dt.uint32
I32 = mybir.dt.int32
ALU = mybir.AluOpType
ACTF = mybir.ActivationFunctionType

N_CORES = 8
TOKENS = 65536
M = 1024
E = 64          # experts per gate
EC = 2 * E      # concat [gate | noise-gate]
TPC = TOKENS // N_CORES   # tokens per core = 8192
ST_T = 512      # tokens per supertile
N_ST = TPC // ST_T        # 16 supertiles
N_SUB = ST_T // 128       # 4 token sub-tiles per supertile
N_MC = M // 128           # 8 contraction chunks

_CACHED_NC = None

# All ACT funcs we use (Copy/Identity/Exp/Ln) live together in the
# "natural_log_exp_and_others" table. The default greedy table picker
# ping-pongs between exp_and_others and natural_log (one table load per
# activation, ~200us of pure overhead), so restrict eligibility to the
# single table that covers everything. Positions (= act_func_set_id)
# must be preserved, hence empty sets rather than a filtered dict.
_ONLY_TABLE = "natural_log_exp_and_others"


def _patched_act_tables(arch):
    import concourse.hw_specs as hw_specs
    full = hw_specs.get_activation_tables(arch)
    return {name: (fns if name == _ONLY_TABLE else set())
            for name, fns in full.items()}


class _act_table_patch:
    """Scoped patch of bacc's table picker (restored immediately after
    compile); position/IDs of the tables are preserved."""

    def __enter__(self):
        self._orig = bacc.get_activation_tables
        bacc.get_activation_tables = _patched_act_tables

    def __exit__(self, *exc):
        bacc.get_activation_tables = self._orig


# tuning knobs (TimelineSim-swept)
XIN_BUFS = 2
XT_BUFS = 2
VEC_BUFS = 2
PS_XT_BUFS = 3
OUT_DMA = "gpsimd"   # gpsimd | sync | scalar


def _build_nc(repeat=1):
    nc = bacc.Bacc("TRN2", target_bir_lowering=False, debug=False,
                   num_devices=N_CORES)
    x_h = nc.dram_tensor("x", [TPC, M], F32, kind="ExternalInput")
    noise_h = nc.dram_tensor("noise", [TPC, E], F32, kind="ExternalInput")
    wcat_h = nc.dram_tensor("wcat", [M, EC], F32, kind="ExternalInput")
    bcat_h = nc.dram_tensor("bcat", [EC, 1], F32, kind="ExternalInput")
    gates_h = nc.dram_tensor("gates", [TPC, E], F32, kind="ExternalOutput")
    hout_h = nc.dram_tensor("hout", [TPC, E], F32, kind="ExternalOutput")
    idx_h = nc.dram_tensor("topk_idx", [TPC, 2], I32, kind="ExternalOutput")
    ns_h = nc.dram_tensor("noise_scale", [TPC, E], F32, kind="ExternalOutput")
    logits_h = nc.dram_tensor("logits", [TPC, E], F32, kind="ExternalOutput")

    with tile.TileContext(nc) as tc:
        with (
            tc.tile_pool(name="const", bufs=1) as constp,
            tc.tile_pool(name="xin", bufs=XIN_BUFS) as xinp,
            tc.tile_pool(name="xt", bufs=XT_BUFS) as xtp,
            tc.tile_pool(name="noise", bufs=2) as noisep,
            tc.tile_pool(name="ltsb", bufs=2) as ltp,
            tc.tile_pool(name="vec", bufs=VEC_BUFS) as vecp,
            tc.tile_pool(name="stats", bufs=1) as statsp,
            tc.tile_pool(name="ps_xt", bufs=PS_XT_BUFS, space="PSUM") as ps_xt,
            tc.tile_pool(name="ps_lt", bufs=2, space="PSUM") as ps_lt,
            tc.tile_pool(name="ps_ht", bufs=2, space="PSUM") as ps_ht,
        ):
            ident = constp.tile([128, 128], F32)
            masks.make_identity(nc, ident[:])
            wcat = constp.tile([128, N_MC, EC], F32)
            nc.sync.dma_start(
                wcat[:], wcat_h.ap().rearrange("(c p) e -> p c e", p=128))
            bcat = constp.tile([EC, 1], F32)
            nc.sync.dma_start(bcat[:], bcat_h.ap())

            # core-wide stats tiles (one column-group of 8 per 128-token tile)
            max8S = statsp.tile([128, N_ST * N_SUB, 8], F32)
            idx8S = statsp.tile([128, N_ST * N_SUB, 8], U32)
            biasS = statsp.tile([128, N_ST * N_SUB], F32)
            idx_i32 = statsp.tile([128, N_ST * N_SUB, 2], I32)

            x_dram = x_h.ap().rearrange("(st s p) m -> st p s m", p=128, s=N_SUB)
            noise_dram = noise_h.ap().rearrange(
                "(st s p) e -> st p s e", p=128, s=N_SUB)

            def out_view(h):
                return h.ap().rearrange("(st s p) e -> st p s e", p=128, s=N_SUB)

            gates_dram = out_view(gates_h)
            hout_dram = out_view(hout_h)
            ns_dram = out_view(ns_h)
            logits_dram = out_view(logits_h)

            import contextlib
            loop_cm = (tc.For_i(0, repeat, 1) if repeat > 1
                       else contextlib.nullcontext())
            with loop_cm:
                _emit_body(nc, tc, locals())

    with _act_table_patch():
        nc.compile()
    return nc


def _emit_body(nc, tc, env):
    ident = env["ident"]; wcat = env["wcat"]; bcat = env["bcat"]
    max8S = env["max8S"]; idx8S = env["idx8S"]; biasS = env["biasS"]
    idx_i32 = env["idx_i32"]
    x_dram = env["x_dram"]; noise_dram = env["noise_dram"]
    gates_dram = env["gates_dram"]; hout_dram = env["hout_dram"]
    ns_dram = env["ns_dram"]; logits_dram = env["logits_dram"]
    idx_h = env["idx_h"]
    xinp = env["xinp"]; xtp = env["xtp"]; noisep = env["noisep"]
    ltp = env["ltp"]; vecp = env["vecp"]
    ps_xt = env["ps_xt"]; ps_lt = env["ps_lt"]; ps_ht = env["ps_ht"]
    if True:
        if True:
            for st in range(N_ST):
                # ---- load x supertile ----
                x_sb = xinp.tile([128, N_SUB, M], F32, tag="x_sb")
                nc.sync.dma_start(x_sb[:], x_dram[st])

                noise_sb = noisep.tile([128, N_SUB, E], F32, tag="noise_sb")
                nc.sync.dma_start(noise_sb[:], noise_dram[st])

                # ---- transpose x -> xT [m, t] chunks ----
                xt_sb = xtp.tile([128, N_MC, ST_T], F32, tag="xt_sb")
                for mc in range(N_MC):
                    pxt = ps_xt.tile([128, ST_T], F32, tag="pxt")
                    for s in range(N_SUB):
                        nc.tensor.transpose(
                            pxt[:, s * 128:(s + 1) * 128],
                            x_sb[:, s, mc * 128:(mc + 1) * 128],
                            ident[:])
                    eng = nc.vector if mc % 2 == 0 else nc.scalar
                    if eng is nc.vector:
                        nc.vector.tensor_copy(xt_sb[:, mc, :], pxt[:])
                    else:
                        nc.scalar.copy(xt_sb[:, mc, :], pxt[:])

                # ---- gate matmuls: psum[ec, t] += Wcat[mc].T @ xT[mc] ----
                plt = ps_lt.tile([128, ST_T], F32, tag="plt")
                for mc in range(N_MC):
                    nc.tensor.matmul(
                        plt[:], wcat[:, mc, :], xt_sb[:, mc, :],
                        start=(mc == 0), stop=(mc == N_MC - 1))

                # ---- bias add during PSUM->SBUF copy ----
                lt_sb = ltp.tile([128, ST_T], F32, tag="lt_sb")
                nc.scalar.activation(lt_sb[:], plt[:], ACTF.Identity,
                                     bias=bcat[:])

                # ---- transpose back to [t, ec] ----
                pht = ps_ht.tile([128, N_SUB, 128], F32, tag="pht")
                for s in range(N_SUB):
                    nc.tensor.transpose(
                        pht[:, s, :],
                        lt_sb[:, s * 128:(s + 1) * 128],
                        ident[:])

                # ---- batched activations over the supertile ----
                esp = vecp.tile([128, N_SUB, E], F32, tag="esp")
                ns_sb = vecp.tile([128, N_SUB, E], F32, tag="ns_sb")
                logit_sb = vecp.tile([128, N_SUB, E], F32, tag="logit_sb")
                nc.scalar.activation(esp[:], pht[:, :, E:EC], ACTF.Exp)
                nc.scalar.activation(ns_sb[:], esp[:], ACTF.Ln, bias=1.0)
                nc.vector.tensor_copy(logit_sb[:], pht[:, :, 0:E])

                tmp = vecp.tile([128, N_SUB, E], F32, tag="tmp")
                nc.vector.tensor_mul(tmp[:], noise_sb[:], ns_sb[:])
                h_sb = vecp.tile([128, N_SUB, E], F32, tag="h_sb")
                nc.vector.tensor_add(h_sb[:], tmp[:], logit_sb[:])

                # ---- top-8 ----
                for s in range(N_SUB):
                    g = st * N_SUB + s
                    nc.vector.max(max8S[:, g, :], h_sb[:, s, :])
                    nc.vector.max_index(idx8S[:, g, :], max8S[:, g, :],
                                        h_sb[:, s, :])

                # ---- softmax bias: -(max1 + ln(1+exp(max2-max1))) ----
                g0 = st * N_SUB
                max1 = max8S[:, g0:g0 + N_SUB, 0:1]
                max2 = max8S[:, g0:g0 + N_SUB, 1:2]
                d_t = vecp.tile([128, N_SUB], F32, tag="d_t")
                nc.vector.tensor_sub(
                    d_t[:].rearrange("p (s o) -> p s o", o=1), max2, max1)
                e_t = vecp.tile([128, N_SUB], F32, tag="e_t")
                nc.scalar.activation(e_t[:], d_t[:], ACTF.Exp)
                l_t = vecp.tile([128, N_SUB], F32, tag="l_t")
                nc.scalar.activation(l_t[:], e_t[:], ACTF.Ln, bias=1.0)
                s_t = vecp.tile([128, N_SUB], F32, tag="s_t")
                nc.vector.tensor_add(
                    s_t[:].rearrange("p (s o) -> p s o", o=1),
                    l_t[:].rearrange("p (s o) -> p s o", o=1), max1)
                nc.vector.tensor_scalar_mul(biasS[:, g0:g0 + N_SUB], s_t[:], -1.0)

                # ---- gates = Exp(H + bias) * (H >= max2) ----
                mg = vecp.tile([128, N_SUB, E], F32, tag="mg")
                expt = vecp.tile([128, N_SUB, E], F32, tag="expt")
                for s in range(N_SUB):
                    g = st * N_SUB + s
                    nc.vector.tensor_scalar(
                        mg[:, s, :], h_sb[:, s, :],
                        scalar1=max8S[:, g, 1:2], scalar2=None, op0=ALU.is_ge)
                    nc.scalar.activation(expt[:, s, :], h_sb[:, s, :], ACTF.Exp,
                                         bias=biasS[:, g:g + 1])
                gates_sb = vecp.tile([128, N_SUB, E], F32, tag="gates_sb")
                nc.vector.tensor_mul(gates_sb[:], expt[:], mg[:])

                # ---- outputs ----
                out_eng = {"gpsimd": nc.gpsimd, "sync": nc.sync,
                           "scalar": nc.scalar}[OUT_DMA]
                out_eng.dma_start(gates_dram[st], gates_sb[:])
                out_eng.dma_start(hout_dram[st], h_sb[:])
                out_eng.dma_start(ns_dram[st], ns_sb[:])
                out_eng.dma_start(logits_dram[st], logit_sb[:])

            # ---- topk_idx: cast u32->i32, one DMA ----
            nc.vector.tensor_copy(idx_i32[:], idx8S[:, :, 0:2])
            nc.sync.dma_start(
                idx_h.ap().rearrange("(a p) b -> p a b", p=128), idx_i32[:])


def _get_nc():
    global _CACHED_NC
    if _CACHED_NC is None:
        _CACHED_NC = _build_nc()
    return _CACHED_NC


def run(x, noise, Wg_w, Wg_b, Wn_w, Wn_b, **spmd_kwargs):
    x = np.ascontiguousarray(np.asarray(x, dtype=np.float32))
    noise = np.ascontiguousarray(np.asarray(noise, dtype=np.float32))
    wcat = np.ascontiguousarray(
        np.concatenate([np.asarray(Wg_w).T, np.asarray(Wn_w).T], axis=1)
    ).astype(np.float32)
    bcat = np.concatenate(
        [np.asarray(Wg_b), np.asarray(Wn_b)]).astype(np.float32).reshape(EC, 1)

    nc = _get_nc()
    in_maps = []
    for c in range(N_CORES):
        sl = slice(c * TPC, (c + 1) * TPC)
        in_maps.append({
            "x": x[sl],
            "noise": noise[sl],
            "wcat": wcat,
            "bcat": bcat,
        })
    res = run_bass_kernel_spmd(nc, in_maps, core_ids=list(range(N_CORES)),
                               **spmd_kwargs)
    gates = np.concatenate([r["gates"] for r in res.results], axis=0)
    hout = np.concatenate([r["hout"] for r in res.results], axis=0)
    topk_idx = np.concatenate([r["topk_idx"] for r in res.results], axis=0)
    ns = np.concatenate([r["noise_scale"] for r in res.results], axis=0)
    logits = np.concatenate([r["logits"] for r in res.results], axis=0)
    return (gates, hout, topk_idx.astype(np.int32), ns, logits), res


def kernel(x, noise, Wg_w, Wg_b, Wn_w, Wn_b):
    outs, _ = run(x, noise, Wg_w, Wg_b, Wn_w, Wn_b)
    return outs
